# revision 1
# baseline (speedup 1.0000x reference)
"""Trainium2 Bass kernel for nn_CrossAttnBlockpp (cross-attention block).

Sharding: data-parallel over the 8 image pairs (B//2 = 8), one pair per
NeuronCore. Inside each core: group norm -> q/k/v 1x1 projections ->
pairwise cross attention (4 heads) -> output projection + residual.

All large matmuls run in float32r (fp32 with mantissa rounded to 11 bits,
full PE rate at N>=256). Softmax skips the max-subtraction (scores are
~N(0,1); exp is fp32-safe) so attention can be computed entirely in the
k^T q orientation with column-sum denominators from a ones matmul.
"""
import os
import sys

for _p in ("/opt/trn_rl_repo", "/root/.axon_site/_ro/trn_rl_repo"):
    if _p not in sys.path and os.path.isdir(_p):
        sys.path.append(_p)

import numpy as np
import concourse.bass as bass
import concourse.bacc as bacc
import concourse.tile as tile
from concourse import mybir
from concourse.bass_utils import run_bass_kernel_spmd

f32 = mybir.dt.float32
f32r = mybir.dt.float32r
AF = mybir.ActivationFunctionType
ALU = mybir.AluOpType

B, C, COND, HEADS, H, W = 16, 128, 32, 4, 32, 32
HW = H * W                      # 1024
NPAIR = B // 2                  # 8 cores, one pair each
GROUPS = 32
GSIZE = C // GROUPS             # 4 channels per group
NORM_N = GSIZE * HW             # elements per group
EPS = 1e-6
SCALE = float(C) ** -0.5


def round_f32r(x: np.ndarray) -> np.ndarray:
    """Round fp32 to the fp32r grid (11-bit mantissa) like the HW expects."""
    u = np.ascontiguousarray(x, dtype=np.float32).view(np.uint32)
    r = (u.astype(np.uint64) + 0x800) & 0xFFFFF000
    return r.astype(np.uint32).view(np.float32)


def _build_program(reps: int = 1, bcast: str = "gpsimd"):
    nc = bacc.Bacc("TRN2", target_bir_lowering=False, debug=False,
                   num_devices=NPAIR)

    def din(name, shape, dt):
        return nc.dram_tensor(name, shape, dt, kind="ExternalInput").ap()

    x2 = din("x2", (2, C, HW), f32)
    qc = din("qc", (COND, HW), f32r)
    kca = din("kca", (COND, HW), f32r)
    kcb = din("kcb", (COND, HW), f32r)
    onesrow = din("onesrow", (1, HW), f32r)
    onescol = din("onescol", (C, 2), f32r)
    wqh = din("wqh", (C, 512), f32r)
    wkh = din("wkh", (C, 512), f32r)
    wvh = din("wvh", (C, 512), f32r)
    wqc = din("wqc", (COND + 1, 512), f32r)
    wkc = din("wkc", (COND + 1, 512), f32r)
    wvc = din("wvc", (COND + 1, 512), f32r)
    w3c = din("w3c", (512, C), f32r)
    b3t = din("b3t", (C, 1), f32)
    gns = din("gns", (C, 1), f32)
    gnb = din("gnb", (C, 1), f32)
    Gi = din("Gi", (C, GROUPS), f32)
    GTi = din("GTi", (GROUPS, C), f32)
    out = nc.dram_tensor("out", (2, C, HW), f32, kind="ExternalOutput").ap()

    with tile.TileContext(nc) as tc:
        with tc.tile_pool(name="const", bufs=1) as cpool, \
             tc.tile_pool(name="img", bufs=1) as ipool, \
             tc.tile_pool(name="work", bufs=2) as wpool, \
             tc.tile_pool(name="small", bufs=2) as spool, \
             tc.tile_pool(name="psum", bufs=2, space="PSUM") as pspool, \
             tc.tile_pool(name="dram", bufs=2, space="DRAM") as dpool:

            # ---- constants / weights into SBUF ----
            def cload(name, ap, shape, dt):
                t = cpool.tile(shape, dt, tag=name)
                nc.sync.dma_start(out=t, in_=ap)
                return t

            # x first (group norm is the critical path), then GN consts,
            # cond tiles + small weight tails (early PE work), then the rest
            xs, hs, qs, ks, vTs = [], [], [], [], []
            for i in range(2):
                xs.append(ipool.tile([C, HW], f32, tag=f"x{i}", name=f"x{i}"))
                nc.sync.dma_start(out=xs[i], in_=x2[i])
                hs.append(ipool.tile([C, HW], f32r, tag=f"h{i}", name=f"h{i}"))
                qs.append(ipool.tile([C, HEADS * HW], f32r, tag=f"q{i}", name=f"q{i}"))
                ks.append(ipool.tile([C, HEADS * HW], f32r, tag=f"k{i}", name=f"k{i}"))
                vTs.append(ipool.tile([C, 8 * 512], f32r, tag=f"vT{i}", name=f"vT{i}"))

            t_gns = cload("gns", gns, [C, 1], f32)
            t_gnb = cload("gnb", gnb, [C, 1], f32)
            t_G = cload("G", Gi, [C, GROUPS], f32)
            t_GT = cload("GT", GTi, [GROUPS, C], f32)
            t_ones2 = cload("ones2", onescol, [C, 2], f32r)

            conds = {}
            for name, ap in (("qc", qc), ("kca", kca), ("kcb", kcb)):
                t = cpool.tile([COND + 1, HW], f32r, tag=f"cond_{name}")
                nc.sync.dma_start(out=t[0:COND, :], in_=ap)
                nc.sync.dma_start(out=t[COND:COND + 1, :], in_=onesrow)
                conds[name] = t

            t_wqc = cload("wqc", wqc, [COND + 1, 512], f32r)
            t_wkc = cload("wkc", wkc, [COND + 1, 512], f32r)
            t_wvc = cload("wvc", wvc, [COND + 1, 512], f32r)
            t_wqh = cload("wqh", wqh, [C, 512], f32r)
            t_wkh = cload("wkh", wkh, [C, 512], f32r)
            t_wvh = cload("wvh", wvh, [C, 512], f32r)
            t_b3 = cload("b3", b3t, [C, 1], f32)

            t_w3 = cpool.tile([C, 4 * C], f32r, tag="w3")
            for h in range(HEADS):
                nc.sync.dma_start(out=t_w3[:, h * C:(h + 1) * C],
                                  in_=w3c[h * C:(h + 1) * C, :])

            t_eps = cpool.tile([GROUPS, 1], f32, tag="eps")
            nc.vector.memset(t_eps, EPS)

            # ---- group norm (per image) ----
            def run_gn():
              for i in range(2):
                  s2 = spool.tile([C, 2], f32, tag="gn_s2")
                  nc.vector.reduce_sum(out=s2[:, 0:1], in_=xs[i],
                                       axis=mybir.AxisListType.X)
                  sqout = wpool.tile([C, HW], f32, tag="bc")
                  nc.scalar.activation(out=sqout, in_=xs[i], func=AF.Square,
                                       accum_out=s2[:, 1:2])
                  ps_g = pspool.tile([GROUPS, 2], f32, tag="den", bufs=1)
                  nc.tensor.matmul(ps_g, t_G, s2, start=True, stop=True)
                  sb_g = spool.tile([GROUPS, 2], f32, tag="gn_g")
                  nc.scalar.mul(out=sb_g, in_=ps_g, mul=1.0 / NORM_N)
                  var = spool.tile([GROUPS, 1], f32, tag="gn_var")
                  # var = E[x^2] - mean^2
                  nc.vector.tensor_mul(out=var, in0=sb_g[:, 0:1], in1=sb_g[:, 0:1])
                  nc.vector.tensor_sub(out=var, in0=sb_g[:, 1:2], in1=var)
                  nc.scalar.activation(out=var, in_=var, func=AF.Sqrt, bias=t_eps)
                  rstd = spool.tile([GROUPS, 1], f32, tag="gn_rstd")
                  nc.vector.reciprocal(out=rstd, in_=var)
                  stats2 = spool.tile([GROUPS, 2], f32, tag="gn_stats2")
                  nc.vector.tensor_copy(out=stats2[:, 0:1], in_=sb_g[:, 0:1])
                  nc.vector.tensor_copy(out=stats2[:, 1:2], in_=rstd)
                  ps_bc = pspool.tile([C, 2], f32, tag="den", bufs=1)
                  nc.tensor.matmul(ps_bc, t_GT, stats2, start=True, stop=True)
                  # s_c = rstd_c * gn_scale_c ; t_c = gn_bias_c - mean_c * s_c
                  s_c = spool.tile([C, 1], f32, tag="gn_sc")
                  t_c = spool.tile([C, 1], f32, tag="gn_tc")
                  nc.vector.tensor_mul(out=s_c, in0=ps_bc[:, 1:2], in1=t_gns)
                  nc.vector.tensor_mul(out=t_c, in0=ps_bc[:, 0:1], in1=s_c)
                  nc.vector.tensor_sub(out=t_c, in0=t_gnb, in1=t_c)
                  nc.vector.tensor_scalar(out=hs[i], in0=xs[i], scalar1=s_c,
                                          scalar2=t_c, op0=ALU.mult, op1=ALU.add)

            # ---- projections ----
            def project_qk(dst, wh, wc, himg, cond):
                # dst[cout, px] = [wh; wc]^T [h; cond; 1]  (per head chunk)
                for m in range(HEADS):
                    ps = pspool.tile([C, HW], f32, tag="big")
                    for nh in range(2):
                        sl = slice(nh * 512, (nh + 1) * 512)
                        nc.tensor.matmul(ps[:, sl], wc[:, m * C:(m + 1) * C],
                                         cond[:, sl], start=True, stop=False)
                        nc.tensor.matmul(ps[:, sl], wh[:, m * C:(m + 1) * C],
                                         himg[:, sl], start=False, stop=True)
                    nc.vector.tensor_copy(out=dst[:, m * HW:(m + 1) * HW], in_=ps)

            def project_vT(dst, himg, cond):
                # dst[px, cout] = X^T W  : lhsT = X px-slices, rhs = W chunks
                for j in range(8):
                    sl = slice(j * C, (j + 1) * C)
                    ps = pspool.tile([C, 512], f32, tag="big")
                    nc.tensor.matmul(ps, cond[:, sl], t_wvc, start=True, stop=False)
                    nc.tensor.matmul(ps, himg[:, sl], t_wvh, start=False, stop=True)
                    nc.vector.tensor_copy(out=dst[:, j * 512:(j + 1) * 512], in_=ps)

            def run_gn_proj():
                run_gn()
                for i in range(2):
                    kc = conds["kca"] if i == 0 else conds["kcb"]
                    project_qk(qs[i], t_wqh, t_wqc, hs[i], conds["qc"])
                    project_qk(ks[i], t_wkh, t_wkc, hs[i], kc)
                    project_vT(vTs[i], hs[i], kc)

            run_gn_proj()

            # ---- attention units + final projection ----
            att = {}   # (img, head) -> normalized [C, HW] f32r tile

            def unit(qi, h):
                """Image qi attends to keys/values of the other image."""
                ki = 1 - qi
                q_t, k_t, vT_t = qs[qi], ks[ki], vTs[ki]
                ps_att = pspool.tile([C, HW], f32, tag="att", bufs=1)
                ps_den = pspool.tile([2, HW], f32, tag="den", bufs=1)
                for c8 in range(8):
                    ps_s = pspool.tile([C, HW], f32, tag="big")
                    for nh in range(2):
                        sl = slice(nh * 512, (nh + 1) * 512)
                        nc.tensor.matmul(
                            ps_s[:, sl],
                            k_t[:, h * HW + c8 * C: h * HW + (c8 + 1) * C],
                            q_t[:, h * HW + nh * 512: h * HW + (nh + 1) * 512],
                            start=True, stop=True)
                    ex = wpool.tile([C, HW], f32r, tag="exp", bufs=3)
                    nc.scalar.activation(out=ex, in_=ps_s, func=AF.Exp,
                                         scale=SCALE)
                    for nh in range(2):
                        sl = slice(nh * 512, (nh + 1) * 512)
                        nc.tensor.matmul(ps_den[:, sl], t_ones2, ex[:, sl],
                                         start=(c8 == 0), stop=(c8 == 7))
                        nc.tensor.matmul(
                            ps_att[:, sl],
                            vT_t[:, c8 * 512 + h * C: c8 * 512 + (h + 1) * C],
                            ex[:, sl],
                            start=(c8 == 0), stop=(c8 == 7))
                a = wpool.tile([C, HW], f32r, tag="attn", bufs=5)
                if bcast == "none":
                    # timing-only variant: skip normalization entirely
                    nc.vector.reciprocal(
                        out=wpool.tile([1, HW], f32, tag="recip", bufs=1,
                                       name="recip"),
                        in_=ps_den[0:1, :])
                    nc.vector.tensor_copy(out=a, in_=ps_att)
                elif bcast == "gpsimd":
                    recip = wpool.tile([1, HW], f32, tag="recip", bufs=1)
                    nc.vector.reciprocal(out=recip, in_=ps_den[0:1, :])
                    bc = wpool.tile([C, HW], f32, tag="bc")
                    nc.gpsimd.partition_broadcast(bc, recip)
                    attU = wpool.tile([C, HW], f32, tag="attU")
                    nc.vector.tensor_copy(out=attU, in_=ps_att)
                    nc.vector.tensor_mul(out=a, in0=attU, in1=bc)
                else:
                    recip = wpool.tile([1, HW], f32, tag="recip", bufs=1)
                    nc.vector.reciprocal(out=recip, in_=ps_den[0:1, :])
                    rdram = dpool.tile([1, HW], f32, tag="recip_d", name="rdram")
                    nc.sync.dma_start(out=rdram, in_=recip)
                    bc = wpool.tile([C, HW], f32, tag="bc")
                    bcast_ap = rdram.partition_broadcast(C).squeeze(1)
                    nc.sync.dma_start(out=bc[:, 0:512], in_=bcast_ap[:, 0:512])
                    nc.sync.dma_start(out=bc[:, 512:HW], in_=bcast_ap[:, 512:HW])
                    attU = wpool.tile([C, HW], f32, tag="attU")
                    nc.vector.tensor_copy(out=attU, in_=ps_att)
                    nc.vector.tensor_mul(out=a, in0=attU, in1=bc)
                att[(qi, h)] = a

            def final(i):
                ps_f = pspool.tile([C, HW], f32, tag="big")
                for nh in range(2):
                    sl = slice(nh * 512, (nh + 1) * 512)
                    for h in range(HEADS):
                        nc.tensor.matmul(ps_f[:, sl],
                                         t_w3[:, h * C:(h + 1) * C],
                                         att[(i, h)][:, sl],
                                         start=(h == 0), stop=(h == HEADS - 1))
                o = wpool.tile([C, HW], f32, tag="out")
                # o = (ps_f + b3) + x
                nc.vector.scalar_tensor_tensor(out=o, in0=ps_f, scalar=t_b3,
                                               in1=xs[i], op0=ALU.add,
                                               op1=ALU.add)
                nc.sync.dma_start(out=out[i], in_=o)

            for _rep in range(reps):
                if _rep > 0:
                    for i in range(2):
                        nc.sync.dma_start(out=xs[i], in_=x2[i])
                    run_gn_proj()
                unit(0, 0); unit(0, 1); unit(0, 2); unit(0, 3)
                unit(1, 0); unit(1, 1)
                final(0)
                unit(1, 2); unit(1, 3)
                final(1)
                att.clear()

    nc.compile()
    return nc


_CACHE = {}


def _get_program():
    if "nc" not in _CACHE:
        _CACHE["nc"] = _build_program()
    return _CACHE["nc"]


def make_in_maps(x, q_cond, k_a_cond, k_b_cond, gn_scale, gn_bias,
                 W0, b0, W1, b1, W2, b2, W3, b3):
    x = np.asarray(x, np.float32)
    r = round_f32r
    eye = np.eye(GROUPS, dtype=np.float32)
    Gi = np.repeat(eye, GSIZE, axis=0)            # (C, GROUPS)
    GTi = np.ascontiguousarray(Gi.T)              # (GROUPS, C)
    onesrow = np.ones((1, HW), np.float32)
    onescol = np.zeros((C, 2), np.float32)
    onescol[:, 0] = 1.0

    def wsplit(Wm, bm):
        head = r(np.ascontiguousarray(Wm[:C]))
        tailc = np.concatenate([Wm[C:], np.asarray(bm)[None, :]], axis=0)
        return head, r(np.ascontiguousarray(tailc))

    wqh_, wqc_ = wsplit(np.asarray(W0, np.float32), b0)
    wkh_, wkc_ = wsplit(np.asarray(W1, np.float32), b1)
    wvh_, wvc_ = wsplit(np.asarray(W2, np.float32), b2)
    w3c_ = r(np.asarray(W3, np.float32))

    shared = {
        "onesrow": r(onesrow), "onescol": r(onescol),
        "wqh": wqh_, "wkh": wkh_, "wvh": wvh_,
        "wqc": wqc_, "wkc": wkc_, "wvc": wvc_,
        "w3c": w3c_,
        "b3t": np.asarray(b3, np.float32).reshape(C, 1),
        "gns": np.asarray(gn_scale, np.float32).reshape(C, 1),
        "gnb": np.asarray(gn_bias, np.float32).reshape(C, 1),
        "Gi": Gi, "GTi": GTi,
    }
    in_maps = []
    for p in range(NPAIR):
        m = dict(shared)
        m["x2"] = np.ascontiguousarray(x[2 * p:2 * p + 2].reshape(2, C, HW))
        m["qc"] = r(np.asarray(q_cond[p], np.float32).reshape(COND, HW))
        m["kca"] = r(np.asarray(k_a_cond[p], np.float32).reshape(COND, HW))
        m["kcb"] = r(np.asarray(k_b_cond[p], np.float32).reshape(COND, HW))
        in_maps.append(m)
    return in_maps


def kernel(**inputs) -> np.ndarray:
    nc = _get_program()
    in_maps = make_in_maps(**inputs)
    res = run_bass_kernel_spmd(nc, in_maps, core_ids=list(range(NPAIR)))
    outs = [res.results[p]["out"].reshape(2, C, H, W) for p in range(NPAIR)]
    return np.concatenate(outs, axis=0).astype(np.float32)

